# revision 1
# baseline (speedup 1.0000x reference)
# Trainium2 Bass kernel for DistNSA (sparse attention, 3 branches).
#
# Strategy: shard the 2048 queries contiguously across 8 NeuronCores (256
# queries each); K/V are replicated to every core.  On each core everything is
# computed in a "transposed" E-layout [keys(part), queries(free)] so the PV
# matmuls need no on-chip transposition of the probability matrices:
#   - window branch: E_win = exp(L) * win01 mask (host-precomputed, position-only)
#   - compressed branch: computed [q(part), blocks(free)] so the softmax
#     denominator falls out of the activation accum and the top-k runs on the
#     free axis (nc.vector.max/max_index reproduce jax.lax.top_k tie-breaks)
#   - selected branch: selection mask expanded blocks->keys via a small matmul
#     and multiplied with a causal host mask; E_slt = exp(L) * selcaus01
# Denominators for the transposed branches come from all-ones-lhsT matmuls
# (which broadcast the per-query sums to all 128 partitions for free);
# normalization uses reciprocal_approx_accurate (~2 ULP).
import numpy as np

import concourse.bass as bass
import concourse.bacc as bacc_mod
import concourse.mybir as mybir
from concourse.tile import TileContext

F32 = mybir.dt.float32
F32R = mybir.dt.float32r
BF16 = mybir.dt.bfloat16
U32 = mybir.dt.uint32
AOT = mybir.ActivationFunctionType
ALU = mybir.AluOpType

S, NHQ, NHK, HD = 2048, 8, 2, 128
REP = NHQ // NHK
WIN, BLK, NB, TOPN = 512, 32, 64, 4
SCALE = float(HD) ** -0.5
NCORE = 8
SQ = S // NCORE          # 256 queries per core
NKT = S // 128           # 16 key tiles
NEG_EPS = 1e-30


def _r(ap):
    return ap.bitcast(F32R)


def build_nc() -> bass.Bass:
    import os
    PHASE = int(os.environ.get("NSA_PHASE", "6"))
    nc = bacc_mod.Bacc("TRN2", target_bir_lowering=False, debug=False)

    # ---------------- DRAM I/O ----------------
    qT_d = nc.dram_tensor("qT", [NHQ, HD, SQ], F32R, kind="ExternalInput")
    qTf_d = nc.dram_tensor("qTf", [NHQ, HD, SQ], F32, kind="ExternalInput")
    kT_d = nc.dram_tensor("kT", [NHK, HD, S], F32R, kind="ExternalInput")
    vb_d = nc.dram_tensor("vb", [NHK, NKT, 128, HD], BF16, kind="ExternalInput")
    bm_d = nc.dram_tensor("bm", [NKT, 128, NB], BF16, kind="ExternalInput")
    win_d = nc.dram_tensor("win01T", [128, NKT * SQ], BF16, kind="ExternalInput")
    caus_d = nc.dram_tensor("caus01T", [128, NKT * SQ], BF16, kind="ExternalInput")
    nval_d = nc.dram_tensor("nvalid", [128, 2], F32, kind="ExternalInput")
    negc_d = nc.dram_tensor("negc", [128, 2, NB], F32, kind="ExternalInput")
    bon_d = nc.dram_tensor("bonus", [128, 2, NB], F32, kind="ExternalInput")
    io64_d = nc.dram_tensor("iota64", [128, NB], F32, kind="ExternalInput")
    grow_d = nc.dram_tensor("grow", [1, 2 * NHQ * SQ], F32, kind="ExternalInput")
    gcq_d = nc.dram_tensor("gcq", [128, 2, NHQ], F32, kind="ExternalInput")
    idf_d = nc.dram_tensor("identf", [128, 128], F32, kind="ExternalInput")
    idb_d = nc.dram_tensor("identb", [128, 128], BF16, kind="ExternalInput")
    on128_d = nc.dram_tensor("ones128", [128, 128], BF16, kind="ExternalInput")
    ex01_d = nc.dram_tensor("expand01", [NB, NKT * 128], BF16, kind="ExternalInput")
    oT_d = nc.dram_tensor("oT", [NHQ, HD, SQ], F32, kind="ExternalOutput")
    dbg_d = nc.dram_tensor("dbg", [NHK, 2, 128, 8], F32, kind="ExternalOutput")

    from contextlib import ExitStack

    with TileContext(nc) as tc, ExitStack() as ctx:
        cpool = ctx.enter_context(tc.tile_pool(name="const", bufs=1))
        wpool = ctx.enter_context(tc.tile_pool(name="work", bufs=2))
        epool = ctx.enter_context(tc.tile_pool(name="espace", bufs=1))
        ps = ctx.enter_context(tc.tile_pool(name="ps", bufs=2, space="PSUM"))
        psL = ctx.enter_context(tc.tile_pool(name="psL", bufs=1, space="PSUM"))
        psPV = ctx.enter_context(tc.tile_pool(name="psPV", bufs=1, space="PSUM"))

        def scratch(shape, dtype=F32, name="scr_ps"):
            return ps.tile(shape, dtype, name=name, tag="ps_scratch")

        # ------------- persistent loads -------------
        qT = cpool.tile([128, NHQ, SQ], F32R, name="qT_s")
        nc.sync.dma_start(qT, qT_d.rearrange("h d q -> d h q"))
        qTf = cpool.tile([128, NHQ, SQ], F32, name="qTf_s")
        nc.sync.dma_start(qTf, qTf_d.rearrange("h d q -> d h q"))
        kT = cpool.tile([128, NHK, S], F32R, name="kT_s")
        nc.sync.dma_start(kT, kT_d.rearrange("g d k -> d g k"))
        vb = cpool.tile([128, NHK, NKT, HD], BF16, name="vb_s")
        nc.sync.dma_start(vb, vb_d.rearrange("g t k d -> k g t d"))
        bm = cpool.tile([128, NKT, NB], BF16, name="bm_s")
        nc.sync.dma_start(bm, bm_d.rearrange("t k n -> k t n"))
        win01 = cpool.tile([128, NKT * SQ], BF16, name="win_s")
        nc.sync.dma_start(win01, win_d[:])
        caus01 = cpool.tile([128, NKT * SQ], BF16, name="caus_s")
        nc.sync.dma_start(caus01, caus_d[:])
        nval = cpool.tile([128, 2], F32, name="nval_s")
        nc.sync.dma_start(nval, nval_d[:])
        negc = cpool.tile([128, 2, NB], F32, name="negc_s")
        nc.sync.dma_start(negc, negc_d[:])
        bon = cpool.tile([128, 2, NB], F32, name="bon_s")
        nc.sync.dma_start(bon, bon_d[:])
        io64 = cpool.tile([128, NB], F32, name="io64_s")
        nc.sync.dma_start(io64, io64_d[:])
        grow = cpool.tile([1, 2 * NHQ * SQ], F32, name="grow_s")
        nc.sync.dma_start(grow, grow_d[:])
        gcq = cpool.tile([128, 2, NHQ], F32, name="gcq_s")
        nc.sync.dma_start(gcq, gcq_d[:])
        idf = cpool.tile([128, 128], F32, name="idf_s")
        nc.sync.dma_start(idf, idf_d[:])
        idb = cpool.tile([128, 128], BF16, name="idb_s")
        nc.sync.dma_start(idb, idb_d[:])
        on128 = cpool.tile([128, 128], BF16, name="on128_s")
        nc.sync.dma_start(on128, on128_d[:])
        ex01 = cpool.tile([NB, NKT * 128], BF16, name="ex01_s")
        nc.sync.dma_start(ex01, ex01_d[:])

        # computed persistents
        kcT = cpool.tile([128, NHK, NB], F32, name="kcT_s")
        vcs = cpool.tile([NB, NHK, HD], BF16, name="vcs_s")
        rcs = cpool.tile([128, NHK, 2, REP], F32, name="rcs_s")

        # block means: kcT via pooled average over kT columns; vc via bm matmul
        for g in range(NHK):
            nc.vector.tensor_reduce(
                out=kcT[:, g],
                in_=kT[:, g].rearrange("p (n b) -> p n b", b=BLK).bitcast(F32),
                axis=mybir.AxisListType.X, op=ALU.add,
            )
            nc.vector.tensor_scalar_mul(kcT[:, g], kcT[:, g], 1.0 / BLK)
            vcp = scratch([NB, HD], name="vcp")
            for kt in range(NKT):
                nc.tensor.matmul(
                    vcp, bm[:, kt], vb[:, g, kt],
                    start=(kt == 0), stop=(kt == NKT - 1),
                )
            nc.vector.tensor_copy(vcs[:, g], vcp)

        for g in range(NHK):
            if PHASE < 2:
                continue
            # ---------------- cmp branch + selection ----------------
            ecT = wpool.tile([NB, REP, SQ], BF16, name="ecT", tag="ecT")
            bTs = wpool.tile([NB, SQ], BF16, name="bTs", tag="bTs")
            for qh in range(2):
                qsl = slice(qh * 128, (qh + 1) * 128)
                pg = [
                    wpool.tile([128, NB], F32, name=f"pg{i}", tag=f"pg{i}")
                    for i in range(2)
                ]
                for r in range(REP):
                    h = g * REP + r
                    lc = scratch([128, NB], name="lc")
                    nc.tensor.matmul(lc, qTf[:, h, qsl], kcT[:, g])
                    lcm = wpool.tile([128, NB], F32, name="lcm", tag="lcm")
                    nc.vector.scalar_tensor_tensor(
                        out=lcm, in0=lc, scalar=SCALE,
                        in1=negc[:, qh], op0=ALU.mult, op1=ALU.add,
                    )
                    ec = wpool.tile([128, NB], F32, name="ec", tag="ec")
                    zc = wpool.tile([128, 1], F32, name="zc", tag="zc")
                    nmx = wpool.tile([128, 1], F32, name="nmx", tag="nmx")
                    nc.vector.tensor_reduce(
                        out=nmx, in_=lcm, axis=mybir.AxisListType.X,
                        op=ALU.max, negate=True,
                    )
                    nc.scalar.activation(ec, lcm, AOT.Exp, bias=nmx, accum_out=zc)
                    nc.vector.tensor_scalar_add(zc, zc, NEG_EPS)
                    nc.vector.reciprocal(rcs[:, g, qh, r:r + 1], zc)
                    if r == 0:
                        nc.vector.tensor_scalar(
                            pg[0], ec, rcs[:, g, qh, r:r + 1], None, op0=ALU.mult
                        )
                    else:
                        nc.vector.scalar_tensor_tensor(
                            out=pg[r % 2], in0=ec, scalar=rcs[:, g, qh, r:r + 1],
                            in1=pg[(r + 1) % 2], op0=ALU.mult, op1=ALU.add,
                        )
                    # stash E_cmp^T (bf16) for the cmp PV
                    ecb = wpool.tile([128, NB], BF16, name="ecb", tag="ecb")
                    nc.vector.tensor_copy(ecb, ec)
                    ectp = scratch([NB, 128], BF16, name="ectp")
                    nc.tensor.transpose(ectp, ecb, idb)
                    nc.vector.tensor_copy(ecT[:, r, qsl], ectp)
                # selection: score = pg + bonus; exact top-4 (ties -> low idx)
                score = wpool.tile([128, NB], F32, name="score", tag="score")
                nc.vector.tensor_add(score, pg[(REP - 1) % 2], bon[:, qh])
                mx8 = wpool.tile([128, 8], F32, name="mx8", tag="mx8")
                nc.vector.max(out=mx8, in_=score)
                ix8 = wpool.tile([128, 8], U32, name="ix8", tag="ix8")
                nc.vector.max_index(ix8, mx8, score)
                ixf = wpool.tile([128, TOPN], F32, name="ixf", tag="ixf")
                nc.vector.tensor_copy(ixf, ix8[:, :TOPN])
                bsel = [
                    wpool.tile([128, NB], BF16, name=f"bsel{i}", tag=f"bsel{i}")
                    for i in range(2)
                ]
                nc.vector.tensor_scalar(
                    bsel[0], io64, ixf[:, 0:1], None, op0=ALU.is_equal
                )
                for t in range(1, TOPN):
                    nc.vector.scalar_tensor_tensor(
                        out=bsel[t % 2], in0=io64, scalar=ixf[:, t:t + 1],
                        in1=bsel[(t + 1) % 2], op0=ALU.is_equal, op1=ALU.add,
                    )
                dbgrow = wpool.tile([128, 8], F32, name="dbgrow", tag="dbgrow")
                nc.vector.tensor_copy(dbgrow, ix8)
                nc.sync.dma_start(dbg_d[g, qh], dbgrow)
                btp = scratch([NB, 128], BF16, name="btp")
                nc.tensor.transpose(btp, bsel[(TOPN - 1) % 2], idb)
                nc.vector.tensor_copy(bTs[:, qsl], btp)

            if PHASE < 3:
                continue
            # ---------------- selection mask expand to keys ----------------
            selc = epool.tile([128, NKT * SQ], BF16, name="selc", tag="selc")
            for kt in range(NKT):
                bex = scratch([128, SQ], name="bex")
                nc.tensor.matmul(
                    bex, ex01[:, kt * 128:(kt + 1) * 128], bTs
                )
                nc.vector.scalar_tensor_tensor(
                    out=selc[:, kt * SQ:(kt + 1) * SQ], in0=bex, scalar=1.0,
                    in1=caus01[:, kt * SQ:(kt + 1) * SQ],
                    op0=ALU.mult, op1=ALU.mult,
                )

            if PHASE < 4:
                continue
            # ---------------- main QK + exp + masks ----------------
            ew = [
                epool.tile([128, NKT * SQ], BF16, name=f"ew{r}", tag=f"ew{r}")
                for r in range(REP)
            ]
            es = [
                epool.tile([128, NKT * SQ], BF16, name=f"es{r}", tag=f"es{r}")
                for r in range(REP)
            ]
            for hp in range(2):          # head pairs
                for grp in range(8):     # groups of 2 key tiles
                    lsp = [
                        psL.tile([128, 2 * SQ], F32, name=f"lsp{hh}")
                        for hh in range(2)
                    ]
                    for kt4 in range(2):
                        kt = grp * 2 + kt4
                        for hh in range(2):
                            r = hp * 2 + hh
                            h = g * REP + r
                            nc.tensor.matmul(
                                lsp[hh][:, kt4 * SQ:(kt4 + 1) * SQ],
                                kT[:, g, kt * 128:(kt + 1) * 128],
                                qT[:, h],
                            )
                    gsl = slice(grp * 2 * SQ, (grp + 1) * 2 * SQ)
                    for hh in range(2):
                        r = hp * 2 + hh
                        esp = wpool.tile([128, 2 * SQ], BF16, name="esp", tag="esp")
                        nc.scalar.activation(esp, lsp[hh], AOT.Exp, scale=SCALE)
                        nc.vector.tensor_mul(ew[r][:, gsl], esp, win01[:, gsl])
                        nc.vector.tensor_mul(es[r][:, gsl], esp, selc[:, gsl])

            if PHASE < 5:
                continue
            # ------- PV + Z + normalize + combine, per head pair -------
            for hp in range(2):
              opvw = [
                  psPV.tile([128, SQ], F32, name=f"opvw{hh}", tag=f"opvw{hh}")
                  for hh in range(2)
              ]
              opvs = [
                  psPV.tile([128, SQ], F32, name=f"opvs{hh}", tag=f"opvs{hh}")
                  for hh in range(2)
              ]
              for kt in range(NKT):
                ksl = slice(kt * SQ, (kt + 1) * SQ)
                for hh in range(2):
                    r = hp * 2 + hh
                    nc.tensor.matmul(
                        opvw[hh], vb[:, g, kt], ew[r][:, ksl],
                        start=(kt == 0), stop=(kt == NKT - 1),
                    )
                    nc.tensor.matmul(
                        opvs[hh], vb[:, g, kt], es[r][:, ksl],
                        start=(kt == 0), stop=(kt == NKT - 1),
                    )
              if PHASE < 6:
                  continue
              for hh in range(2):
                r = hp * 2 + hh
                h = g * REP + r
                acc = wpool.tile([128, SQ], F32, name="acc", tag="acc")
                tmp = wpool.tile([128, SQ], F32, name="tmpc", tag="tmpc")
                for br in range(2):
                    esrc = ew[r] if br == 0 else es[r]
                    zbc = scratch([128, SQ], name="zbc")
                    for kt in range(NKT):
                        nc.tensor.matmul(
                            zbc, on128, esrc[:, kt * SQ:(kt + 1) * SQ],
                            start=(kt == 0), stop=(kt == NKT - 1),
                        )
                    zsb = wpool.tile([128, SQ], F32, name="zsb", tag="zsb")
                    nc.vector.tensor_copy(zsb, zbc)
                    rz = wpool.tile([128, SQ], F32, name="rz", tag="rz")
                    scr = wpool.tile([128, SQ], F32, name="scr", tag="scr")
                    nc.vector.reciprocal_approx_accurate(out=rz, in_=zsb, scratch=scr)
                    # gains broadcast [1,SQ] -> [128,SQ]
                    gb = wpool.tile([128, SQ], F32, name="gb", tag="gb")
                    nc.gpsimd.partition_broadcast(
                        gb, grow[:, (br * NHQ + h) * SQ:(br * NHQ + h + 1) * SQ]
                    )
                    gr = wpool.tile([128, SQ], F32, name="gr", tag="gr")
                    nc.vector.tensor_mul(gr, gb, rz)
                    if br == 0:
                        pvt = wpool.tile([128, SQ], F32, name="pvt", tag="pvt")
                        nc.vector.tensor_mul(pvt, opvw[hh], gr)
                        nc.vector.tensor_copy(acc, pvt)
                    else:
                        nc.vector.tensor_mul(tmp, opvs[hh], gr)
                        nc.vector.tensor_add(acc, acc, tmp)
                # cmp branch: PV + per-partition normalize, then transpose-add
                for qh in range(2):
                    qsl = slice(qh * 128, (qh + 1) * 128)
                    ocp = scratch([128, HD], name="ocp")
                    nc.tensor.matmul(ocp, ecT[:, r, qsl], vcs[:, g])
                    gcr = wpool.tile([128, 1], F32, name="gcr", tag="gcr")
                    nc.vector.tensor_mul(
                        gcr, gcq[:, qh, h:h + 1], rcs[:, g, qh, r:r + 1]
                    )
                    ocs = wpool.tile([128, HD], BF16, name="ocs", tag="ocs")
                    nc.vector.tensor_scalar(ocs, ocp, gcr, None, op0=ALU.mult)
                    octp = scratch([128, 128], BF16, name="octp")
                    nc.tensor.transpose(octp, ocs, idb)
                    nc.vector.tensor_add(acc[:, qsl], acc[:, qsl], octp)
                nc.sync.dma_start(oT_d[h], acc)

        if PHASE < 6:
            for h in range(NHQ):
                accz = wpool.tile([128, SQ], F32, name="accz", tag="accz")
                nc.vector.memset(accz, 0.0)
                nc.sync.dma_start(oT_d[h], accz)

    nc.finalize()
    return nc


# ------------------------- host side -------------------------

def _host_inputs(core: int, q, k, v, g_win, g_cmp, g_slt):
    q_off = core * SQ
    qc = q[q_off:q_off + SQ]                       # [SQ, 8, 128]
    s_glob = np.arange(q_off, q_off + SQ)
    kpos = np.arange(S)

    dif = s_glob[None, :] - kpos[:, None]          # [S(key), SQ(q)]
    win01 = ((dif >= 0) & (dif < WIN)).astype(np.float32)
    caus01 = (dif >= 0).astype(np.float32)
    # [S, SQ] -> [128, NKT*SQ] with column = kt*SQ + jq
    win01T = win01.reshape(NKT, 128, SQ).transpose(1, 0, 2).reshape(128, NKT * SQ)
    caus01T = caus01.reshape(NKT, 128, SQ).transpose(1, 0, 2).reshape(128, NKT * SQ)

    nvalid = np.minimum((s_glob + 1) // BLK, NB).astype(np.float32)
    bonus = np.zeros((SQ, NB), np.float32)
    bonus[np.arange(SQ), np.minimum(s_glob // BLK, NB - 1)] += 1e6
    bonus[:, 0] += 1e6

    bmat = np.zeros((S, NB), np.float32)
    bmat[np.arange(S), kpos // BLK] = 1.0 / BLK
    ex01 = np.zeros((NB, S), np.float32)
    ex01[kpos // BLK, np.arange(S)] = 1.0

    grow = np.stack([g_win[q_off:q_off + SQ].T, g_slt[q_off:q_off + SQ].T])

    qT_host = np.ascontiguousarray(qc.transpose(1, 2, 0))
    return {
        "qT": qT_host,
        "qTf": qT_host,
        "kT": np.ascontiguousarray(k.transpose(1, 2, 0)),
        "vb": np.ascontiguousarray(
            v.transpose(1, 0, 2).reshape(NHK, NKT, 128, HD)
        ).astype(np.float32).astype(_bf16()),
        "bm": bmat.reshape(NKT, 128, NB).astype(_bf16()),
        "win01T": win01T.astype(_bf16()),
        "caus01T": caus01T.astype(_bf16()),
        "nvalid": np.ascontiguousarray(nvalid.reshape(2, 128).T),
        "negc": np.ascontiguousarray(
            np.where(
                np.arange(NB)[None, :] < nvalid[:, None], 0.0, -1e30
            ).astype(np.float32).reshape(2, 128, NB).transpose(1, 0, 2)
        ),
        "bonus": np.ascontiguousarray(
            bonus.reshape(2, 128, NB).transpose(1, 0, 2)
        ),
        "iota64": np.broadcast_to(
            np.arange(NB, dtype=np.float32), (128, NB)
        ).copy(),
        "grow": grow.reshape(1, 2 * NHQ * SQ).astype(np.float32).copy(),
        "gcq": np.ascontiguousarray(
            (g_cmp[q_off:q_off + SQ] * (nvalid > 0)[:, None])
            .reshape(2, 128, NHQ).transpose(1, 0, 2)
        ),
        "identf": np.eye(128, dtype=np.float32),
        "identb": np.eye(128, dtype=np.float32).astype(_bf16()),
        "ones128": np.ones((128, 128), np.float32).astype(_bf16()),
        "expand01": ex01.astype(_bf16()),
    }


def _bf16():
    import ml_dtypes
    return ml_dtypes.bfloat16


_CACHE = {}


def kernel(q, k, v, g_win, g_cmp, g_slt):
    q = np.asarray(q, np.float32)
    k = np.asarray(k, np.float32)
    v = np.asarray(v, np.float32)
    g_win = np.asarray(g_win, np.float32)
    g_cmp = np.asarray(g_cmp, np.float32)
    g_slt = np.asarray(g_slt, np.float32)

    from concourse.bass_utils import run_bass_kernel_spmd

    if "nc" not in _CACHE:
        _CACHE["nc"] = build_nc()
    nc = _CACHE["nc"]

    in_maps = [
        _host_inputs(c, q, k, v, g_win, g_cmp, g_slt) for c in range(NCORE)
    ]
    import os
    res = run_bass_kernel_spmd(
        nc, in_maps, core_ids=list(range(NCORE)),
        trace=bool(int(os.environ.get("NSA_TRACE", "0"))),
    )
    out = np.empty((S, NHQ, HD), np.float32)
    for c in range(NCORE):
        oT = res.results[c]["oT"]                  # [8, 128, 256]
        out[c * SQ:(c + 1) * SQ] = oT.transpose(2, 0, 1)
    _CACHE["last_result"] = res
    return out

